# revision 23
# baseline (speedup 1.0000x reference)
"""Trainium2 Bass kernel for nn_FALayer (gnn_message_passing).

Reference computation (N=4096 nodes, D=512, H=8 heads, dh=64):
    q = (X[sub] @ Wq.T + bq) ; k = (X[obj] @ Wk.T + bk)     per edge
    scores[e,h] = <q_h, k_h> / 8
    af[H,N,N]: scores scattered at edges, -inf where agg==0, diag:=1e-7
    p = softmax(af, axis=2); out = (p.sum(0)/H) @ X

Restructure used here (per local row i, with AE = edge&agg (diag off),
M0 = agg' - AE, V_h = AE o exp(S_h), r_h = rowsum(V_h) + rowsum(M0)):
    out = C @ (X/8),  C[i,j] = M0[i,j]*Sum_h(1/r_h[i])
                              + Sum_h V_h[i,j]/r_h[i]
i.e. all H heads and the base term collapse into ONE [R,N]@[N,D]
matmul instead of 8, at the cost of building C once.  Scores are
computed in [i-partition, j-free] orientation so the per-row 1/r and
rowsum are cheap per-partition ops; C is then transposed tile-wise on
the tensor engine for the final matmul.

Per-core engine pipeline (rows sharded, 512 rows/core, blocks of 128):
    PE:   QK projections -> score matmuls -> C transposes -> C @ X/8
    ScE:  exp on score PSUM (1024-wide chunks) -> bf16 SBUF, proj bias
    DVE:  scalar_tensor_tensor V = exp o AE with rowsum accum_out;
          per-head C += inv_h*V_h as tensor_scalar(4x) + tensor_add(2x)
          (fused STT runs 1x on this RTL; TS/TT bf16 hit fast modes,
          but any op carrying accum_out drops to 1x)
No collectives; masks/rowsums sliced per core on the host.  GpSimd is
left idle on purpose: its tensor_scalar measures ~9 G elem/s here.
"""

import numpy as np
import ml_dtypes

import concourse.bass as bass
import concourse.mybir as mybir
import concourse.tile as tile
from concourse.bass_utils import run_bass_kernel_spmd

N = 4096
D = 512
H = 8
DH = 64
NCORES = 8
R = N // NCORES  # 512 rows per core
P = 128
IT = R // P  # 4 i-blocks per core
DT = D // P  # 4 d-tiles
JC = N // 1024  # 4 exp chunks of 1024
NT = N // P  # 32 j-tiles for the final matmul

BF16 = ml_dtypes.bfloat16
F32 = mybir.dt.float32
BF = mybir.dt.bfloat16
F8 = mybir.dt.float8e4
F8NP = mybir.dt.np(F8)


def _split_waits(nc, limit=1):
    """neuronxcc here accepts at most one semaphore wait per instruction;
    move excess waits onto preceding NoOps on the same in-order queue."""
    n = 0
    for f in nc.m.functions:
        for bb in f.blocks:
            new = []
            for inst in bb.instructions:
                si = getattr(inst, "sync_info", None)
                waits = list(si.on_wait) if si is not None and si.on_wait else []
                if (
                    len(waits) > limit
                    and not isinstance(inst, mybir.InstAllEngineBarrier)
                    and inst.engine is not None
                    and inst.engine != mybir.EngineType.Unassigned
                ):
                    excess, keep = waits[:-limit], waits[-limit:]
                    for i in range(0, len(excess), limit):
                        new.append(
                            mybir.InstNoOp(
                                name=nc.get_next_instruction_name(),
                                sync_info=mybir.SyncInfo(
                                    on_wait=excess[i : i + limit], on_update=[]
                                ),
                                bass_nofuse=True,
                                engine=inst.engine,
                            )
                        )
                        n += 1
                    si.on_wait = keep
                new.append(inst)
            bb.instructions[:] = new
    return n


def _build_nc(split=True):
    nc = bass.Bass()
    Act = mybir.ActivationFunctionType
    Alu = mybir.AluOpType

    # ---- DRAM I/O (host pre-tiled layouts) ----
    x8_t = nc.dram_tensor("x8_t", [P, NT, D], BF, kind="ExternalInput")
    xt_t = nc.dram_tensor("xt_t", [DT, P, N], BF, kind="ExternalInput")
    wkt_t = nc.dram_tensor("wkt_t", [P, DT, D], BF, kind="ExternalInput")
    wqt_t = nc.dram_tensor("wqt_t", [P, DT, D], BF, kind="ExternalInput")
    bk_t = nc.dram_tensor("bk_t", [P, DT], F32, kind="ExternalInput")
    bq_t = nc.dram_tensor("bq_t", [P, DT], F32, kind="ExternalInput")
    xtloc_t = nc.dram_tensor("xtloc_t", [P, DT, R], BF, kind="ExternalInput")
    ae_t = nc.dram_tensor("ae_t", [P, IT, N], BF, kind="ExternalInput")
    m0_t = nc.dram_tensor("m0_t", [P, IT, N], F8, kind="ExternalInput")
    rb_t = nc.dram_tensor("rb_t", [P, IT], F32, kind="ExternalInput")
    id_t = nc.dram_tensor("id_t", [P, P], BF, kind="ExternalInput")
    out_t = nc.dram_tensor("out_t", [P, IT, D], F32, kind="ExternalOutput")

    with tile.TileContext(nc) as tc:
        with (
            tc.tile_pool(name="const", bufs=1) as const,
            tc.tile_pool(name="aep", bufs=2) as aep,
            tc.tile_pool(name="m0p", bufs=1) as m0p,
            tc.tile_pool(name="vcb", bufs=2) as vcb,
            tc.tile_pool(name="ep", bufs=3) as ep,
            tc.tile_pool(name="cacc", bufs=2) as cacc,
            tc.tile_pool(name="ctp", bufs=1) as ctp,
            tc.tile_pool(name="outp", bufs=2) as outp,
            tc.tile_pool(name="smalls", bufs=24) as smalls,
            tc.tile_pool(name="spsum", bufs=3, space="PSUM") as spsum,
            tc.tile_pool(name="tpsum", bufs=1, space="PSUM") as tpsum,
            tc.tile_pool(name="gpsum", bufs=1, space="PSUM") as gpsum,
        ):
            # ---- resident tiles ----
            kt_sb = const.tile([P, DT, N], BF)
            qt_sb = const.tile([P, DT, R], BF)
            x8_sb = const.tile([P, NT, D], BF)
            id_sb = const.tile([P, P], BF)
            bk_sb = const.tile([P, DT], F32)
            bq_sb = const.tile([P, DT], F32)
            rb_sb = const.tile([P, IT], F32)
            bq8_sb = const.tile([P, DT], F32)

            ae_sb = [None] * IT
            m0_sb = [None] * IT

            with tc.tile_pool(name="proj", bufs=1) as projp:
                wkt_sb = projp.tile([P, DT, D], BF)
                for m in range(DT):
                    nc.sync.dma_start(
                        out=wkt_sb[:, m, :], in_=wkt_t[:][:, m, :]
                    )
                nc.sync.dma_start(out=bk_sb, in_=bk_t[:])
                nc.sync.dma_start(out=bq_sb, in_=bq_t[:])
                nc.scalar.mul(out=bq8_sb, in_=bq_sb, mul=0.125)
                nc.sync.dma_start(out=rb_sb, in_=rb_t[:])
                nc.sync.dma_start(out=id_sb, in_=id_t[:])
                wqt_sb = projp.tile([P, DT, D], BF)
                nc.sync.dma_start(out=wqt_sb, in_=wqt_t[:])
                xtloc_sb = projp.tile([P, DT, R], BF)
                nc.sync.dma_start(out=xtloc_sb, in_=xtloc_t[:])
                xt_sb = projp.tile([P, DT, N], BF)
                for t in range(DT):
                    nc.sync.dma_start(
                        out=xt_sb[:, t, 0:2048], in_=xt_t[:][t][:, 0:2048]
                    )
                for t in range(DT):
                    nc.sync.dma_start(
                        out=xt_sb[:, t, 2048:N], in_=xt_t[:][t][:, 2048:N]
                    )
                # masks and x8 queued behind the projection inputs
                ae_sb[0] = aep.tile([P, N], BF, name="ae")
                nc.sync.dma_start(out=ae_sb[0], in_=ae_t[:][:, 0, :])
                ae_sb[1] = aep.tile([P, N], BF, name="ae")
                nc.sync.dma_start(out=ae_sb[1], in_=ae_t[:][:, 1, :])
                m0_sb[0] = m0p.tile([P, N], F8, name="m0")
                nc.sync.dma_start(out=m0_sb[0], in_=m0_t[:][:, 0, :])
                nc.sync.dma_start(out=x8_sb, in_=x8_t[:])

                # KT = Wk @ X.T + bk ; QT = (Wq @ Xloc.T + bq)/8, per
                # 128-row d-slice m (= head pair m): emitted slice by
                # slice so score matmuls can interleave on the PE queue.
                def emit_proj(m, n8s=None, do_qt=True):
                    on_sce = True  # DVE is the bottleneck engine
                    for n8 in n8s if n8s is not None else range(N // 512):
                        ps = spsum.tile([P, 1024], F32, name="s")
                        for t in range(DT):
                            nc.tensor.matmul(
                                ps[:, 0:512],
                                lhsT=wkt_sb[:, t, m * P : (m + 1) * P],
                                rhs=xt_sb[:, t, n8 * 512 : (n8 + 1) * 512],
                                start=(t == 0),
                                stop=(t == DT - 1),
                            )
                        if on_sce:
                            nc.scalar.activation(
                                out=kt_sb[:, m, n8 * 512 : (n8 + 1) * 512],
                                in_=ps[:, 0:512],
                                func=Act.Identity,
                                bias=bk_sb[:, m : m + 1],
                            )
                        else:
                            nc.vector.tensor_scalar_add(
                                out=kt_sb[:, m, n8 * 512 : (n8 + 1) * 512],
                                in0=ps[:, 0:512],
                                scalar1=bk_sb[:, m : m + 1],
                            )
                    if not do_qt:
                        return
                    ps = spsum.tile([P, 1024], F32, name="s")
                    for t in range(DT):
                        nc.tensor.matmul(
                            ps[:, 0:512],
                            lhsT=wqt_sb[:, t, m * P : (m + 1) * P],
                            rhs=xtloc_sb[:, t, :],
                            start=(t == 0),
                            stop=(t == DT - 1),
                        )
                    if on_sce:
                        nc.scalar.activation(
                            out=qt_sb[:, m, :],
                            in_=ps[:, 0:512],
                            func=Act.Identity,
                            bias=bq8_sb[:, m : m + 1],
                            scale=0.125,
                        )
                    else:
                        nc.vector.tensor_scalar(
                            out=qt_sb[:, m, :],
                            in0=ps[:, 0:512],
                            scalar1=bq_sb[:, m : m + 1],
                            scalar2=0.125,
                            op0=Alu.add,
                            op1=Alu.mult,
                        )

                # ---- main pipeline ----
                def pass1_head(b, h, c_sb, v_sb, racc, inv_sb,
                               jcs=None, finish=True):
                    po = (h % 2) * DH
                    hp = h // 2
                    for jc in jcs if jcs is not None else range(JC // 2):
                        e_sb = ep.tile([P, 2048], BF, name="e")
                        for sub in range(2):
                            s_ps = spsum.tile([P, 1024], F32, name="s")
                            for half in range(2):
                                col = jc * 2048 + sub * 1024 + half * 512
                                nc.tensor.matmul(
                                    s_ps[:, half * 512 : (half + 1) * 512],
                                    lhsT=qt_sb[
                                        po : po + DH, hp, b * P : (b + 1) * P
                                    ],
                                    rhs=kt_sb[po : po + DH, hp, col : col + 512],
                                    start=True,
                                    stop=True,
                                )
                            nc.scalar.activation(
                                out=e_sb[:, sub * 1024 : (sub + 1) * 1024],
                                in_=s_ps,
                                func=Act.Exp,
                            )
                        # V = exp(S) o AE with the row-sum as accum side
                        # output.  One DVE pass per 2048 (any op carrying
                        # accum_out runs 1x on this RTL, so fusing
                        # mask+sum is the cheapest form; wide chunks
                        # amortize instruction overhead).
                        nc.vector.scalar_tensor_tensor(
                            out=v_sb[:, jc * 2048 : (jc + 1) * 2048],
                            in0=e_sb,
                            scalar=1.0,
                            in1=ae_sb[b][:, jc * 2048 : (jc + 1) * 2048],
                            op0=Alu.mult,
                            op1=Alu.mult,
                            accum_out=racc[:, jc : jc + 1],
                        )
                    if not finish:
                        return
                    # accum_out of (racc + rb/JC) over the JC cols = full
                    # row denominator in a single small DVE op.
                    rt = smalls.tile([P, 1], F32, name="rt")
                    nc.vector.tensor_scalar(
                        out=racc,
                        in0=racc,
                        scalar1=rb_sb[:, b : b + 1],
                        scalar2=0.0,
                        op0=Alu.add,
                        op1=Alu.add,
                        accum_out=rt,
                    )
                    nc.vector.reciprocal(out=inv_sb[:, h : h + 1], in_=rt)
                    if h == 0:
                        nc.vector.tensor_scalar_mul(
                            out=c_sb, in0=v_sb, scalar1=inv_sb[:, 0:1]
                        )
                    else:
                        nc.vector.tensor_scalar_mul(
                            out=v_sb, in0=v_sb, scalar1=inv_sb[:, h : h + 1]
                        )
                        nc.vector.tensor_add(out=c_sb, in0=c_sb, in1=v_sb)

                def epilogue_a(b, c_sb, inv_sb):
                    # Emitted right at block end (before block b+1's STTs
                    # can overwrite the single C buffer): fold base term,
                    # cast C to bf16.  Returns the bf16 copy.
                    invsum = smalls.tile([P, 1], F32, name="ivs")
                    nc.vector.reduce_sum(
                        out=invsum, in_=inv_sb, axis=mybir.AxisListType.X
                    )
                    nc.vector.scalar_tensor_tensor(
                        out=c_sb,
                        in0=m0_sb[b],
                        scalar=invsum,
                        in1=c_sb,
                        op0=Alu.mult,
                        op1=Alu.add,
                    )
                    if b + 1 < IT:  # prefetch next m0 into the freed buf
                        m0_sb[b + 1] = m0p.tile([P, N], F8, name="m0")
                        nc.sync.dma_start(
                            out=m0_sb[b + 1], in_=m0_t[:][:, b + 1, :]
                        )
                    if b + 2 < IT:
                        ae_sb[b + 2] = aep.tile([P, N], BF, name="ae")
                        nc.sync.dma_start(
                            out=ae_sb[b + 2], in_=ae_t[:][:, b + 2, :]
                        )
                    return c_sb

                def epilogue_b(b, cb_sb):
                    # Deferred (tensor queue): transpose C tiles, then
                    # out_block = C.T.T @ X/8 accumulated over j-tiles.
                    ct_sb = ctp.tile([P, NT, P], BF, name="ct")
                    g_ps = gpsum.tile([P, D], F32, name="g")
                    for g in range(4):
                        ct_ps = tpsum.tile([P, 8, P], BF, name="t")
                        for k in range(8):
                            jt = g * 8 + k
                            nc.tensor.transpose(
                                ct_ps[:, k, :],
                                in_=cb_sb[:, jt * P : (jt + 1) * P],
                                identity=id_sb,
                            )
                        nc.vector.tensor_copy(
                            out=ct_sb[:, g * 8 : (g + 1) * 8, :], in_=ct_ps
                        )
                        for k in range(8):
                            jt = g * 8 + k
                            nc.tensor.matmul(
                                g_ps,
                                lhsT=ct_sb[:, jt, :],
                                rhs=x8_sb[:, jt, :],
                                start=(jt == 0),
                                stop=(jt == NT - 1),
                            )
                    o_sb = outp.tile([P, D], F32, name="o")
                    nc.scalar.copy(out=o_sb, in_=g_ps)
                    nc.sync.dma_start(out=out_t[:][:, b, :], in_=o_sb)

                # Interleave: projection slice m feeds head pair m of
                # block 0; head 0's first chunk starts after only half of
                # KT slice 0 is projected.
                emit_proj(0, n8s=range(4), do_qt=True)
                pend = []  # deferred tensor-side epilogues: (b, cb_sb)

                for b in range(IT):
                    c_sb = cacc.tile([P, N], BF, name="c")
                    inv_sb = smalls.tile([P, H], F32, name="inv")
                    for h in range(H):
                        if b == 0 and h in (2, 4, 6):
                            emit_proj(h // 2)
                        racc = smalls.tile([P, JC // 2], F32, name="racc")
                        v_sb = vcb.tile([P, N], BF, name="v")
                        if b == 0 and h == 0:
                            pass1_head(b, h, c_sb, v_sb, racc, inv_sb,
                                       jcs=(0,), finish=False)
                            emit_proj(0, n8s=range(4, 8), do_qt=False)
                            pass1_head(b, h, c_sb, v_sb, racc, inv_sb,
                                       jcs=(1,), finish=True)
                        else:
                            pass1_head(b, h, c_sb, v_sb, racc, inv_sb)
                        if h == 1 and pend:
                            epilogue_b(*pend.pop())
                    cb_sb = epilogue_a(b, c_sb, inv_sb)
                    pend.append((b, cb_sb))
                epilogue_b(*pend.pop())

    if split:
        _split_waits(nc)
    return nc


_NC_CACHE = None


def _get_nc():
    global _NC_CACHE
    if _NC_CACHE is None:
        _NC_CACHE = _build_nc()
    return _NC_CACHE


def _prep_inputs(inst_feature, aggregator_matrix, rel_pair_index, w_q, b_q, w_k, b_k):
    X = np.asarray(inst_feature, np.float32)
    agg_nz = np.asarray(aggregator_matrix) != 0
    rp = np.asarray(rel_pair_index)
    edge = np.zeros((N, N), dtype=bool)
    edge[rp[:, 0], rp[:, 1]] = True
    diag = np.arange(N)
    ae = edge & agg_nz
    ae[diag, diag] = False
    aggp = agg_nz.copy()
    aggp[diag, diag] = True
    m0 = aggp & ~ae

    def t3(a, tiles):  # [tiles*P, F] -> [P, tiles, F]
        return np.ascontiguousarray(
            a.reshape(tiles, P, a.shape[-1]).transpose(1, 0, 2)
        )

    XT = np.ascontiguousarray(X.T)
    rep = {
        "x8_t": t3((X / 8.0).astype(BF16), NT),
        "xt_t": np.ascontiguousarray(XT.astype(BF16).reshape(DT, P, N)),
        "wkt_t": t3(np.ascontiguousarray(np.asarray(w_k, np.float32).T).astype(BF16), DT),
        "wqt_t": t3(np.ascontiguousarray(np.asarray(w_q, np.float32).T).astype(BF16), DT),
        "bk_t": np.ascontiguousarray(np.asarray(b_k, np.float32).reshape(DT, P).T),
        "bq_t": np.ascontiguousarray(np.asarray(b_q, np.float32).reshape(DT, P).T),
        "id_t": np.eye(P, dtype=BF16),
    }
    in_maps = []
    for c in range(NCORES):
        sl = slice(c * R, (c + 1) * R)
        xloc = np.ascontiguousarray(X[sl].T)  # [D, R]
        rb = m0[sl].sum(axis=1, dtype=np.float32) / (JC // 2)  # [R], pre-divided
        in_maps.append(
            dict(
                rep,
                ae_t=t3(ae[sl].astype(BF16), IT),
                m0_t=t3(m0[sl].astype(F8NP), IT),
                xtloc_t=t3(xloc.astype(BF16), DT),
                rb_t=np.ascontiguousarray(rb.reshape(IT, P).T),
            )
        )
    return in_maps


def run(inputs, trace=False):
    nc = _get_nc()
    in_maps = _prep_inputs(**inputs)
    res = run_bass_kernel_spmd(
        nc, in_maps, core_ids=list(range(NCORES)), trace=trace
    )
    parts = []
    for c in range(NCORES):
        o = res.results[c]["out_t"]  # [P, IT, D]
        parts.append(np.ascontiguousarray(o.transpose(1, 0, 2).reshape(R, D)))
    return np.concatenate(parts, axis=0).astype(np.float32), res


def kernel(**inputs) -> np.ndarray:
    out, _ = run(inputs, trace=False)
    return out


# revision 24
# speedup vs baseline: 1.0109x; 1.0109x over previous
"""Trainium2 Bass kernel for nn_FALayer (gnn_message_passing).

Reference computation (N=4096 nodes, D=512, H=8 heads, dh=64):
    q = (X[sub] @ Wq.T + bq) ; k = (X[obj] @ Wk.T + bk)     per edge
    scores[e,h] = <q_h, k_h> / 8
    af[H,N,N]: scores scattered at edges, -inf where agg==0, diag:=1e-7
    p = softmax(af, axis=2); out = (p.sum(0)/H) @ X

Restructure used here (per local row i, with AE = edge&agg (diag off),
M0 = agg' - AE, V_h = AE o exp(S_h), r_h = rowsum(V_h) + rowsum(M0)):
    out = C @ (X/8),  C[i,j] = M0[i,j]*Sum_h(1/r_h[i])
                              + Sum_h V_h[i,j]/r_h[i]
i.e. all H heads and the base term collapse into ONE [R,N]@[N,D]
matmul instead of 8, at the cost of building C once.  Scores are
computed in [i-partition, j-free] orientation so the per-row 1/r and
rowsum are cheap per-partition ops; C is then transposed tile-wise on
the tensor engine for the final matmul.

Per-core engine pipeline (rows sharded, 512 rows/core, blocks of 128):
    PE:   QK projections -> score matmuls -> C transposes -> C @ X/8
    ScE:  exp on score PSUM (1024-wide chunks) -> bf16 SBUF, proj bias
    DVE:  scalar_tensor_tensor V = exp o AE with rowsum accum_out;
          per-head C += inv_h*V_h as tensor_scalar(4x) + tensor_add(2x)
          (fused STT runs 1x on this RTL; TS/TT bf16 hit fast modes,
          but any op carrying accum_out drops to 1x)
No collectives; masks/rowsums sliced per core on the host.  GpSimd is
left idle on purpose: its tensor_scalar measures ~9 G elem/s here.
"""

import numpy as np
import ml_dtypes

import concourse.bass as bass
import concourse.mybir as mybir
import concourse.tile as tile
from concourse.bass_utils import run_bass_kernel_spmd

N = 4096
D = 512
H = 8
DH = 64
NCORES = 8
R = N // NCORES  # 512 rows per core
P = 128
IT = R // P  # 4 i-blocks per core
DT = D // P  # 4 d-tiles
JC = N // 1024  # 4 exp chunks of 1024
NT = N // P  # 32 j-tiles for the final matmul

BF16 = ml_dtypes.bfloat16
F32 = mybir.dt.float32
BF = mybir.dt.bfloat16
F8 = mybir.dt.float8e4
F8NP = mybir.dt.np(F8)


def _split_waits(nc, limit=1):
    """neuronxcc here accepts at most one semaphore wait per instruction;
    move excess waits onto preceding NoOps on the same in-order queue."""
    n = 0
    for f in nc.m.functions:
        for bb in f.blocks:
            new = []
            for inst in bb.instructions:
                si = getattr(inst, "sync_info", None)
                waits = list(si.on_wait) if si is not None and si.on_wait else []
                if (
                    len(waits) > limit
                    and not isinstance(inst, mybir.InstAllEngineBarrier)
                    and inst.engine is not None
                    and inst.engine != mybir.EngineType.Unassigned
                ):
                    excess, keep = waits[:-limit], waits[-limit:]
                    for i in range(0, len(excess), limit):
                        new.append(
                            mybir.InstNoOp(
                                name=nc.get_next_instruction_name(),
                                sync_info=mybir.SyncInfo(
                                    on_wait=excess[i : i + limit], on_update=[]
                                ),
                                bass_nofuse=True,
                                engine=inst.engine,
                            )
                        )
                        n += 1
                    si.on_wait = keep
                new.append(inst)
            bb.instructions[:] = new
    return n


def _build_nc(split=True):
    nc = bass.Bass()
    Act = mybir.ActivationFunctionType
    Alu = mybir.AluOpType

    # ---- DRAM I/O (host pre-tiled layouts) ----
    x8_t = nc.dram_tensor("x8_t", [P, NT, D], BF, kind="ExternalInput")
    xt_t = nc.dram_tensor("xt_t", [DT, P, N], BF, kind="ExternalInput")
    wkt_t = nc.dram_tensor("wkt_t", [P, DT, D], BF, kind="ExternalInput")
    wqt_t = nc.dram_tensor("wqt_t", [P, DT, D], BF, kind="ExternalInput")
    bk_t = nc.dram_tensor("bk_t", [P, DT], F32, kind="ExternalInput")
    bq_t = nc.dram_tensor("bq_t", [P, DT], F32, kind="ExternalInput")
    xtloc_t = nc.dram_tensor("xtloc_t", [P, DT, R], BF, kind="ExternalInput")
    ae_t = nc.dram_tensor("ae_t", [P, IT, N], BF, kind="ExternalInput")
    m0_t = nc.dram_tensor("m0_t", [P, IT, N], F8, kind="ExternalInput")
    rb_t = nc.dram_tensor("rb_t", [P, IT], F32, kind="ExternalInput")
    id_t = nc.dram_tensor("id_t", [P, P], BF, kind="ExternalInput")
    out_t = nc.dram_tensor("out_t", [P, IT, D], F32, kind="ExternalOutput")

    with tile.TileContext(nc) as tc:
        with (
            tc.tile_pool(name="const", bufs=1) as const,
            tc.tile_pool(name="aep", bufs=2) as aep,
            tc.tile_pool(name="m0p", bufs=1) as m0p,
            tc.tile_pool(name="vcb", bufs=2) as vcb,
            tc.tile_pool(name="ep", bufs=3) as ep,
            tc.tile_pool(name="cacc", bufs=2) as cacc,
            tc.tile_pool(name="ctp", bufs=1) as ctp,
            tc.tile_pool(name="outp", bufs=2) as outp,
            tc.tile_pool(name="smalls", bufs=24) as smalls,
            tc.tile_pool(name="spsum", bufs=3, space="PSUM") as spsum,
            tc.tile_pool(name="tpsum", bufs=1, space="PSUM") as tpsum,
            tc.tile_pool(name="gpsum", bufs=1, space="PSUM") as gpsum,
        ):
            # ---- resident tiles ----
            kt_sb = const.tile([P, DT, N], BF)
            qt_sb = const.tile([P, DT, R], BF)
            x8_sb = const.tile([P, NT, D], BF)
            id_sb = const.tile([P, P], BF)
            bk_sb = const.tile([P, DT], F32)
            bq_sb = const.tile([P, DT], F32)
            rb_sb = const.tile([P, IT], F32)
            bq8_sb = const.tile([P, DT], F32)

            ae_sb = [None] * IT
            m0_sb = [None] * IT

            with tc.tile_pool(name="proj", bufs=1) as projp:
                wkt_sb = projp.tile([P, DT, D], BF)
                for m in range(DT):
                    nc.sync.dma_start(
                        out=wkt_sb[:, m, :], in_=wkt_t[:][:, m, :]
                    )
                nc.sync.dma_start(out=bk_sb, in_=bk_t[:])
                nc.sync.dma_start(out=bq_sb, in_=bq_t[:])
                nc.scalar.mul(out=bq8_sb, in_=bq_sb, mul=0.125)
                nc.sync.dma_start(out=rb_sb, in_=rb_t[:])
                nc.sync.dma_start(out=id_sb, in_=id_t[:])
                wqt_sb = projp.tile([P, DT, D], BF)
                nc.sync.dma_start(out=wqt_sb, in_=wqt_t[:])
                xtloc_sb = projp.tile([P, DT, R], BF)
                nc.sync.dma_start(out=xtloc_sb, in_=xtloc_t[:])
                xt_sb = projp.tile([P, DT, N], BF)
                for t in range(DT):
                    nc.sync.dma_start(
                        out=xt_sb[:, t, 0:2048], in_=xt_t[:][t][:, 0:2048]
                    )
                for t in range(DT):
                    nc.sync.dma_start(
                        out=xt_sb[:, t, 2048:N], in_=xt_t[:][t][:, 2048:N]
                    )
                # masks and x8 queued behind the projection inputs
                ae_sb[0] = aep.tile([P, N], BF, name="ae")
                nc.sync.dma_start(out=ae_sb[0], in_=ae_t[:][:, 0, :])
                ae_sb[1] = aep.tile([P, N], BF, name="ae")
                nc.sync.dma_start(out=ae_sb[1], in_=ae_t[:][:, 1, :])
                m0_sb[0] = m0p.tile([P, N], F8, name="m0")
                nc.sync.dma_start(out=m0_sb[0], in_=m0_t[:][:, 0, :])
                nc.sync.dma_start(out=x8_sb, in_=x8_t[:])

                # KT = Wk @ X.T + bk ; QT = (Wq @ Xloc.T + bq)/8, per
                # 128-row d-slice m (= head pair m): emitted slice by
                # slice so score matmuls can interleave on the PE queue.
                def emit_proj(m, n8s=None, do_qt=True):
                    on_sce = True  # DVE is the bottleneck engine
                    for n8 in n8s if n8s is not None else range(N // 512):
                        ps = spsum.tile([P, 1024], F32, name="s")
                        for t in range(DT):
                            nc.tensor.matmul(
                                ps[:, 0:512],
                                lhsT=wkt_sb[:, t, m * P : (m + 1) * P],
                                rhs=xt_sb[:, t, n8 * 512 : (n8 + 1) * 512],
                                start=(t == 0),
                                stop=(t == DT - 1),
                            )
                        if on_sce:
                            nc.scalar.activation(
                                out=kt_sb[:, m, n8 * 512 : (n8 + 1) * 512],
                                in_=ps[:, 0:512],
                                func=Act.Identity,
                                bias=bk_sb[:, m : m + 1],
                            )
                        else:
                            nc.vector.tensor_scalar_add(
                                out=kt_sb[:, m, n8 * 512 : (n8 + 1) * 512],
                                in0=ps[:, 0:512],
                                scalar1=bk_sb[:, m : m + 1],
                            )
                    if not do_qt:
                        return
                    ps = spsum.tile([P, 1024], F32, name="s")
                    for t in range(DT):
                        nc.tensor.matmul(
                            ps[:, 0:512],
                            lhsT=wqt_sb[:, t, m * P : (m + 1) * P],
                            rhs=xtloc_sb[:, t, :],
                            start=(t == 0),
                            stop=(t == DT - 1),
                        )
                    if on_sce:
                        nc.scalar.activation(
                            out=qt_sb[:, m, :],
                            in_=ps[:, 0:512],
                            func=Act.Identity,
                            bias=bq8_sb[:, m : m + 1],
                            scale=0.125,
                        )
                    else:
                        nc.vector.tensor_scalar(
                            out=qt_sb[:, m, :],
                            in0=ps[:, 0:512],
                            scalar1=bq_sb[:, m : m + 1],
                            scalar2=0.125,
                            op0=Alu.add,
                            op1=Alu.mult,
                        )

                # ---- main pipeline ----
                def pass1_head(b, h, c_sb, v_sb, racc, inv_sb,
                               jcs=None, finish=True):
                    po = (h % 2) * DH
                    hp = h // 2
                    for jc in jcs if jcs is not None else range(JC // 2):
                        e_sb = ep.tile([P, 2048], BF, name="e")
                        for sub in range(2):
                            s_ps = spsum.tile([P, 1024], F32, name="s")
                            for half in range(2):
                                col = jc * 2048 + sub * 1024 + half * 512
                                nc.tensor.matmul(
                                    s_ps[:, half * 512 : (half + 1) * 512],
                                    lhsT=qt_sb[
                                        po : po + DH, hp, b * P : (b + 1) * P
                                    ],
                                    rhs=kt_sb[po : po + DH, hp, col : col + 512],
                                    start=True,
                                    stop=True,
                                )
                            nc.scalar.activation(
                                out=e_sb[:, sub * 1024 : (sub + 1) * 1024],
                                in_=s_ps,
                                func=Act.Exp,
                            )
                        # V = exp(S) o AE with the row-sum as accum side
                        # output.  One DVE pass per 2048 (any op carrying
                        # accum_out runs 1x on this RTL, so fusing
                        # mask+sum is the cheapest form; wide chunks
                        # amortize instruction overhead).
                        nc.vector.scalar_tensor_tensor(
                            out=v_sb[:, jc * 2048 : (jc + 1) * 2048],
                            in0=e_sb,
                            scalar=1.0,
                            in1=ae_sb[b][:, jc * 2048 : (jc + 1) * 2048],
                            op0=Alu.mult,
                            op1=Alu.mult,
                            accum_out=racc[:, jc : jc + 1],
                        )
                    if not finish:
                        return
                    # accum_out of (racc + rb/JC) over the JC cols = full
                    # row denominator in a single small DVE op.
                    rt = smalls.tile([P, 1], F32, name="rt")
                    nc.vector.tensor_scalar(
                        out=racc,
                        in0=racc,
                        scalar1=rb_sb[:, b : b + 1],
                        scalar2=0.0,
                        op0=Alu.add,
                        op1=Alu.add,
                        accum_out=rt,
                    )
                    nc.vector.reciprocal(out=inv_sb[:, h : h + 1], in_=rt)
                    if h == 0:
                        nc.vector.tensor_scalar_mul(
                            out=c_sb, in0=v_sb, scalar1=inv_sb[:, 0:1]
                        )
                    else:
                        nc.vector.tensor_scalar_mul(
                            out=v_sb, in0=v_sb, scalar1=inv_sb[:, h : h + 1]
                        )
                        nc.vector.tensor_add(out=c_sb, in0=c_sb, in1=v_sb)

                def epilogue_a(b, c_sb, inv_sb):
                    # Emitted right at block end (before block b+1's STTs
                    # can overwrite the single C buffer): fold base term,
                    # cast C to bf16.  Returns the bf16 copy.
                    invsum = smalls.tile([P, 1], F32, name="ivs")
                    nc.vector.reduce_sum(
                        out=invsum, in_=inv_sb, axis=mybir.AxisListType.X
                    )
                    nc.vector.scalar_tensor_tensor(
                        out=c_sb,
                        in0=m0_sb[b],
                        scalar=invsum,
                        in1=c_sb,
                        op0=Alu.mult,
                        op1=Alu.add,
                    )
                    if b + 1 < IT:  # prefetch next m0 into the freed buf
                        m0_sb[b + 1] = m0p.tile([P, N], F8, name="m0")
                        nc.sync.dma_start(
                            out=m0_sb[b + 1], in_=m0_t[:][:, b + 1, :]
                        )
                    if b + 2 < IT:
                        ae_sb[b + 2] = aep.tile([P, N], BF, name="ae")
                        nc.sync.dma_start(
                            out=ae_sb[b + 2], in_=ae_t[:][:, b + 2, :]
                        )
                    return c_sb

                def epilogue_b(b, cb_sb):
                    # Deferred (tensor queue): transpose C tiles, then
                    # out_block = C.T.T @ X/8 accumulated over j-tiles.
                    ct_sb = ctp.tile([P, NT, P], BF, name="ct")
                    for g in range(4):
                        ct_ps = tpsum.tile([P, 8, P], BF, name="t")
                        for k in range(8):
                            jt = g * 8 + k
                            nc.tensor.transpose(
                                ct_ps[:, k, :],
                                in_=cb_sb[:, jt * P : (jt + 1) * P],
                                identity=id_sb,
                            )
                        nc.vector.tensor_copy(
                            out=ct_sb[:, g * 8 : (g + 1) * 8, :], in_=ct_ps
                        )
                    g_ps = gpsum.tile([P, D], F32, name="g")
                    for jt in range(NT):
                        nc.tensor.matmul(
                            g_ps,
                            lhsT=ct_sb[:, jt, :],
                            rhs=x8_sb[:, jt, :],
                            start=(jt == 0),
                            stop=(jt == NT - 1),
                        )
                    o_sb = outp.tile([P, D], F32, name="o")
                    nc.scalar.copy(out=o_sb, in_=g_ps)
                    nc.sync.dma_start(out=out_t[:][:, b, :], in_=o_sb)

                # Interleave: projection slice m feeds head pair m of
                # block 0, so ScE starts ~10us in instead of after 40us.
                emit_proj(0)
                pend = []  # deferred tensor-side epilogues: (b, cb_sb)

                for b in range(IT):
                    c_sb = cacc.tile([P, N], BF, name="c")
                    inv_sb = smalls.tile([P, H], F32, name="inv")
                    for h in range(H):
                        if b == 0 and h in (2, 4, 6):
                            emit_proj(h // 2)
                        racc = smalls.tile([P, JC // 2], F32, name="racc")
                        v_sb = vcb.tile([P, N], BF, name="v")
                        pass1_head(b, h, c_sb, v_sb, racc, inv_sb)
                        if h == 1 and pend:
                            epilogue_b(*pend.pop())
                    cb_sb = epilogue_a(b, c_sb, inv_sb)
                    pend.append((b, cb_sb))
                epilogue_b(*pend.pop())

    if split:
        _split_waits(nc)
    return nc


_NC_CACHE = None


def _get_nc():
    global _NC_CACHE
    if _NC_CACHE is None:
        _NC_CACHE = _build_nc()
    return _NC_CACHE


def _prep_inputs(inst_feature, aggregator_matrix, rel_pair_index, w_q, b_q, w_k, b_k):
    X = np.asarray(inst_feature, np.float32)
    agg_nz = np.asarray(aggregator_matrix) != 0
    rp = np.asarray(rel_pair_index)
    edge = np.zeros((N, N), dtype=bool)
    edge[rp[:, 0], rp[:, 1]] = True
    diag = np.arange(N)
    ae = edge & agg_nz
    ae[diag, diag] = False
    aggp = agg_nz.copy()
    aggp[diag, diag] = True
    m0 = aggp & ~ae

    def t3(a, tiles):  # [tiles*P, F] -> [P, tiles, F]
        return np.ascontiguousarray(
            a.reshape(tiles, P, a.shape[-1]).transpose(1, 0, 2)
        )

    XT = np.ascontiguousarray(X.T)
    rep = {
        "x8_t": t3((X / 8.0).astype(BF16), NT),
        "xt_t": np.ascontiguousarray(XT.astype(BF16).reshape(DT, P, N)),
        "wkt_t": t3(np.ascontiguousarray(np.asarray(w_k, np.float32).T).astype(BF16), DT),
        "wqt_t": t3(np.ascontiguousarray(np.asarray(w_q, np.float32).T).astype(BF16), DT),
        "bk_t": np.ascontiguousarray(np.asarray(b_k, np.float32).reshape(DT, P).T),
        "bq_t": np.ascontiguousarray(np.asarray(b_q, np.float32).reshape(DT, P).T),
        "id_t": np.eye(P, dtype=BF16),
    }
    in_maps = []
    for c in range(NCORES):
        sl = slice(c * R, (c + 1) * R)
        xloc = np.ascontiguousarray(X[sl].T)  # [D, R]
        rb = m0[sl].sum(axis=1, dtype=np.float32) / (JC // 2)  # [R], pre-divided
        in_maps.append(
            dict(
                rep,
                ae_t=t3(ae[sl].astype(BF16), IT),
                m0_t=t3(m0[sl].astype(F8NP), IT),
                xtloc_t=t3(xloc.astype(BF16), DT),
                rb_t=np.ascontiguousarray(rb.reshape(IT, P).T),
            )
        )
    return in_maps


def run(inputs, trace=False):
    nc = _get_nc()
    in_maps = _prep_inputs(**inputs)
    res = run_bass_kernel_spmd(
        nc, in_maps, core_ids=list(range(NCORES)), trace=trace
    )
    parts = []
    for c in range(NCORES):
        o = res.results[c]["out_t"]  # [P, IT, D]
        parts.append(np.ascontiguousarray(o.transpose(1, 0, 2).reshape(R, D)))
    return np.concatenate(parts, axis=0).astype(np.float32), res


def kernel(**inputs) -> np.ndarray:
    out, _ = run(inputs, trace=False)
    return out
